# revision 27
# baseline (speedup 1.0000x reference)
"""Multi-head attention (B=2, S=2048, D=1024, H=16, A=64) on 8 TRN2 NeuronCores.

Sharding: core c = b*4 + g handles batch b and head-group g (4 heads).
 - Tensor-parallel over heads; host-side all-reduce of the 4 partial output
   projections per batch during the gather.
 - Key/value sequence is mask-compacted host-side (padded to 128; pad slots
   get an additive -60 bias before exp).

v3 schedule (vs the phase-sequential v1 at ~190us):
 - Both the scalar-engine exp stream (72 x [128,1024] ACTIVATEs ~= 82us) and
   the tensor engine (~89us busy) are near-critical, so emission interleaves
   at sub-microsecond granularity: after each scores+exp pair, a background
   queue doles out ~0.5us of deferred tensor work (projection halves, PV
   thirds, out-projection blocks). Out-projection for the first half of the
   sequence runs while the second half's exps stream.
 - All inputs are host-pre-tiled so every DMA is per-partition contiguous,
   and they load on 3 parallel queues; the first matmul starts ~2us in.
 - Softmax denominator: reciprocal on DVE, partition-broadcast via a DRAM
   round-trip (SBUF->DRAM->SBUF with a 0-stride partition read), the one
   pattern proven reliable on hardware. The PV psum is drained once to SBUF
   so the psum slot frees quickly.
 - PSUM: scores 2x[128,1024] (4 banks) + shared proj/PV pool 2 + outproj 2.
"""

import numpy as np

import concourse.bass as bass
import concourse.bacc as bacc
import concourse.mybir as mybir
import concourse.tile as tile
from concourse.bass_utils import run_bass_kernel_spmd

F32 = mybir.dt.float32
BF16 = mybir.dt.bfloat16
DT_MM = BF16
EXP = mybir.ActivationFunctionType.Exp

B = 2
S = 2048
D = 1024
H = 16
A = 64
HG = 4           # head groups (cores per batch)
HL = H // HG     # heads per core = 4
DSUB = D // 128  # 8
NEG = -60.0      # additive mask bias; exp(-60) vanishes in fp32 sums
QCH = 512
NQC = S // QCH   # 4


def build_program(C: int) -> bass.Bass:
    KT = C // 128
    NKC = (C + QCH - 1) // QCH

    nc = bacc.Bacc("TRN2", target_bir_lowering=False, name=f"mha3_c{C}")
    # host-pre-tiled layouts: every DMA below is per-partition contiguous
    xq_d = nc.dram_tensor("xq", [NQC, 128, DSUB, QCH], DT_MM,
                          kind="ExternalInput")
    xkv_d = nc.dram_tensor("xkv", [NKC, 128, DSUB, QCH], DT_MM,
                           kind="ExternalInput")
    wqkv_d = nc.dram_tensor("wqkv", [3, 128, DSUB, HL * A], DT_MM,
                            kind="ExternalInput")
    wout_d = nc.dram_tensor("wout", [128, 2, D], DT_MM, kind="ExternalInput")
    mb_d = nc.dram_tensor("mbias", [128, KT], F32, kind="ExternalInput")
    out_d = nc.dram_tensor("out", [S, D], DT_MM, kind="ExternalOutput")

    with tile.TileContext(nc) as tc:
        with (
            tc.tile_pool(name="const", bufs=1) as const,
            tc.tile_pool(name="xstrq", bufs=4) as xstrq,
            tc.tile_pool(name="xstrk", bufs=3) as xstrk,
            tc.tile_pool(name="probs", bufs=24) as probs,
            tc.tile_pool(name="norm", bufs=3) as norm,
            tc.tile_pool(name="outp", bufs=11) as outp,
            tc.tile_pool(name="dramp", bufs=4, space="DRAM") as dramp,
            tc.tile_pool(name="psS", bufs=2, space="PSUM") as psS,
            tc.tile_pool(name="psW", bufs=2, space="PSUM") as psW,
            tc.tile_pool(name="psO", bufs=2, space="PSUM") as psO,
        ):
            # ---- persistent SBUF residents ----
            w_sb = const.tile([128, 3, DSUB, HL * A], DT_MM)
            wout_sb = const.tile([128, 2, D], DT_MM)
            mb_sb = const.tile([128, KT], F32)
            qT = const.tile([128, 2, S], DT_MM)       # [hp*64+a, h2, qi]
            kT = const.tile([128, 2, C], DT_MM)       # [hp*64+a, h2, ki]
            vx = const.tile([128, KT, HL, A + 1], DT_MM)  # [ki%128, kt, h, a|1]
            ctxT = const.tile([128, 2, S], DT_MM)     # [hp*64+a, h2, qi]
            ones_col = const.tile([128, KT, HL, 1], F32)

            # ---- parallel DMA kickoff (scalar: weights, sync: xq,
            #      gpsimd: xkv) ----
            xqt = [xstrq.tile([128, DSUB, QCH], DT_MM, tag="xq", name=f"xq{c}")
                   for c in range(NQC)]
            xkt = [xstrk.tile([128, DSUB, QCH], DT_MM, tag="xk", name=f"xk{c}")
                   for c in range(NKC)]
            nc.scalar.dma_start(w_sb[:, 0], wqkv_d.ap()[0])
            nc.sync.dma_start(xqt[0], xq_d.ap()[0])
            nc.gpsimd.dma_start(xkt[0], xkv_d.ap()[0])
            nc.scalar.dma_start(w_sb[:, 1], wqkv_d.ap()[1])
            nc.scalar.dma_start(w_sb[:, 2], wqkv_d.ap()[2])
            nc.scalar.dma_start(mb_sb, mb_d.ap())
            nc.sync.dma_start(xqt[1], xq_d.ap()[1])
            nc.gpsimd.dma_start(xkt[1], xkv_d.ap()[1])
            nc.sync.dma_start(xqt[2], xq_d.ap()[2])
            if NKC > 2:
                nc.gpsimd.dma_start(xkt[2], xkv_d.ap()[2])
            nc.sync.dma_start(xqt[3], xq_d.ap()[3])
            nc.gpsimd.dma_start(wout_sb, wout_d.ap())

            nc.vector.memset(ones_col, 1.0)
            nc.vector.tensor_copy(out=vx[:, :, :, A : A + 1], in_=ones_col)

            prs: dict[tuple, list] = {}

            # ----- background tensor-work queue (thunks of ~0.2-0.9us) ----
            bg: list = []

            def pump(n=1):
                for _ in range(n):
                    if bg:
                        bg.pop(0)()

            def scores(h, kt, half):
                hp, h2 = h % 2, h // 2
                rows = slice(hp * 64, hp * 64 + 64)
                sc = psS.tile([128, 1024], F32, tag="sc",
                              name=f"sc{h}_{kt}_{half}")
                for cc in range(2):
                    q0 = half * 1024 + cc * QCH
                    nc.tensor.matmul(
                        sc[:, cc * QCH : (cc + 1) * QCH],
                        kT[rows, h2, kt * 128 : (kt + 1) * 128],
                        qT[rows, h2, q0 : q0 + QCH],
                        start=True,
                        stop=True,
                    )
                pr = probs.tile([128, 1024], DT_MM, tag="pr",
                                name=f"pr{h}_{kt}_{half}")
                nc.scalar.activation(
                    out=pr, in_=sc, func=EXP,
                    bias=mb_sb[:, kt : kt + 1], scale=1.0,
                )
                prs.setdefault((h, half), []).append(pr)

            _proj_tiles: dict = {}

            def qproj_part(c, ct, lo, hi):
                key = ("q", c, ct)
                if key not in _proj_tiles:
                    _proj_tiles[key] = psW.tile([128, QCH], F32, tag="w",
                                                name=f"qps{c}_{ct}")
                ps = _proj_tiles[key]
                for o in range(lo, hi):
                    nc.tensor.matmul(
                        ps,
                        w_sb[:, 0, o, ct * 128 : (ct + 1) * 128],
                        xqt[c][:, o, :],
                        start=(o == 0),
                        stop=(o == DSUB - 1),
                    )
                if hi == DSUB:
                    nc.vector.tensor_copy(
                        out=qT[:, ct, c * QCH : (c + 1) * QCH], in_=ps
                    )

            def kproj_part(c, ct, lo, hi):
                w = min(QCH, C - c * QCH)
                key = ("k", c, ct)
                if key not in _proj_tiles:
                    _proj_tiles[key] = psW.tile([128, QCH], F32, tag="w",
                                                name=f"kps{c}_{ct}")
                ps = _proj_tiles[key]
                for o in range(lo, hi):
                    nc.tensor.matmul(
                        ps[:, :w],
                        w_sb[:, 1, o, ct * 128 : (ct + 1) * 128],
                        xkt[c][:, o, :w],
                        start=(o == 0),
                        stop=(o == DSUB - 1),
                    )
                if hi == DSUB:
                    nc.vector.tensor_copy(
                        out=kT[:, ct, c * QCH : c * QCH + w], in_=ps[:, :w]
                    )

            def vproj(kt):
                c, k4 = kt // 4, kt % 4
                ps = psW.tile([128, QCH], F32, tag="w", name=f"vps{kt}")
                pvv = ps[:, 0 : HL * A]
                for o in range(DSUB):
                    nc.tensor.matmul(
                        pvv,
                        xkt[c][:, o, k4 * 128 : (k4 + 1) * 128],
                        w_sb[:, 2, o, :],
                        start=(o == 0),
                        stop=(o == DSUB - 1),
                    )
                nc.vector.tensor_copy(
                    out=vx[:, kt, :, 0:A],
                    in_=pvv.rearrange("p (h a) -> p h a", a=A),
                )

            _pv_tiles: dict = {}

            def pv_part(h, c, klo, khi):
                half, sub = c // 2, (c % 2) * QCH
                key = (h, c)
                if key not in _pv_tiles:
                    _pv_tiles[key] = psW.tile([128, QCH], F32, tag="w",
                                              name=f"pv{h}_{c}")
                pv = _pv_tiles[key]
                pl = prs[(h, half)]
                for kt in range(klo, khi):
                    nc.tensor.matmul(
                        pv[0 : A + 1, :],
                        vx[:, kt, h, :],
                        pl[kt][:, sub : sub + QCH],
                        start=(kt == 0),
                        stop=(kt == KT - 1),
                    )

            def norm_unit(h, c):
                hp, h2 = h % 2, h // 2
                pv = _pv_tiles[(h, c)]
                cslice = slice(c * QCH, (c + 1) * QCH)
                ctxr = norm.tile([A + 1, QCH], F32, tag="ctxr")
                nc.vector.tensor_copy(out=ctxr, in_=pv[0 : A + 1, :])
                dnd = dramp.tile([1, QCH], F32, tag="dnd")
                nc.gpsimd.dma_start(dnd, ctxr[A : A + 1, :])
                rBr = norm.tile([64, QCH], F32, tag="rBr")
                dnd_b = bass.AP(
                    tensor=dnd.tensor,
                    offset=dnd.offset,
                    ap=[[0, 64], list(dnd.ap[1])],
                )
                nc.gpsimd.dma_start(rBr, dnd_b)
                rB = norm.tile([64, QCH], F32, tag="rB")
                nc.vector.reciprocal_approx_fast(rB, rBr)
                if hp == 0:
                    nc.vector.tensor_tensor(
                        ctxT[0:64, h2, cslice], ctxr[0:A, :], rB,
                        mybir.AluOpType.mult,
                    )
                else:
                    stg = norm.tile([64, QCH], DT_MM, tag="stg")
                    nc.vector.tensor_tensor(
                        stg, ctxr[0:A, :], rB, mybir.AluOpType.mult
                    )
                    nc.gpsimd.dma_start(ctxT[64:128, h2, cslice], stg)

            def push_pv(h, c):
                bg.append(lambda: pv_part(h, c, 0, 3))
                bg.append(lambda: pv_part(h, c, 3, 6))

                def last():
                    pv_part(h, c, 6, KT)
                    norm_unit(h, c)
                bg.append(last)

            def phase_c(st, dc, dma_eng, ot_holder):
                if dc == 0:
                    ot_holder[st] = outp.tile([128, D], DT_MM, tag="ot",
                                              name=f"ot{st}")
                ot = ot_holder[st]
                po = psO.tile([128, QCH], F32, tag="po", name=f"po{st}_{dc}")
                for s2 in range(2):
                    nc.tensor.matmul(
                        po,
                        ctxT[:, s2, st * 128 : (st + 1) * 128],
                        wout_sb[:, s2, dc * QCH : (dc + 1) * QCH],
                        start=(s2 == 0),
                        stop=(s2 == 1),
                    )
                nc.vector.tensor_copy(
                    out=ot[:, dc * QCH : (dc + 1) * QCH], in_=po
                )
                if dc == 1:
                    dma_eng.dma_start(
                        out_d.ap()[st * 128 : (st + 1) * 128, :], ot
                    )

            _ot: dict = {}

            def push_phase_c(st):
                bg.append(lambda: phase_c(st, 0, nc.sync, _ot))
                bg.append(lambda: phase_c(st, 1, nc.sync, _ot))

            def phase_c_a(st, dc):
                # s2=0 partial (heads 0/1) for the tail sts; runs mid-stream
                if dc == 0:
                    _ot[st] = outp.tile([128, D], DT_MM, tag="ot",
                                        name=f"ot{st}")
                ot = _ot[st]
                po = psO.tile([128, QCH], F32, tag="po", name=f"poa{st}_{dc}")
                nc.tensor.matmul(
                    po,
                    ctxT[:, 0, st * 128 : (st + 1) * 128],
                    wout_sb[:, 0, dc * QCH : (dc + 1) * QCH],
                    start=True,
                    stop=True,
                )
                nc.vector.tensor_copy(
                    out=ot[:, dc * QCH : (dc + 1) * QCH], in_=po
                )

            def phase_c_b(st, dc, dma_eng):
                # s2=1 partial (heads 2/3) + add + store: the exposed tail
                ot = _ot[st]
                po = psO.tile([128, QCH], F32, tag="po", name=f"pob{st}_{dc}")
                nc.tensor.matmul(
                    po,
                    ctxT[:, 1, st * 128 : (st + 1) * 128],
                    wout_sb[:, 1, dc * QCH : (dc + 1) * QCH],
                    start=True,
                    stop=True,
                )
                osl = ot[:, dc * QCH : (dc + 1) * QCH]
                nc.vector.tensor_tensor(osl, po, osl, mybir.AluOpType.add)
                if dc == 1:
                    dma_eng.dma_start(
                        out_d.ap()[st * 128 : (st + 1) * 128, :], ot
                    )

            # ---------------- emission ----------------
            # pre-stream: minimum to unblock h0 half-0 scores
            qproj_part(0, 0, 0, 4)
            qproj_part(0, 0, 4, DSUB)
            qproj_part(1, 0, 0, 4)
            qproj_part(1, 0, 4, DSUB)
            kproj_part(0, 0, 0, 4)
            kproj_part(0, 0, 4, DSUB)
            kproj_part(1, 0, 0, 4)
            kproj_part(1, 0, 4, DSUB)

            # background work, dependency-ordered
            if NKC > 2:
                bg.append(lambda: kproj_part(2, 0, 0, DSUB))
            bg.append(lambda: qproj_part(0, 1, 0, 4))
            bg.append(lambda: qproj_part(0, 1, 4, DSUB))
            bg.append(lambda: qproj_part(1, 1, 0, 4))
            bg.append(lambda: qproj_part(1, 1, 4, DSUB))
            for kt in range(KT):
                bg.append(lambda kt=kt: vproj(kt))
            for c in range(NKC):
                bg.append(lambda c=c: kproj_part(c, 1, 0, 4))
                bg.append(lambda c=c: kproj_part(c, 1, 4, DSUB))
            bg.append(lambda: qproj_part(2, 0, 0, 4))
            bg.append(lambda: qproj_part(2, 0, 4, DSUB))
            bg.append(lambda: qproj_part(3, 0, 0, 4))
            bg.append(lambda: qproj_part(3, 0, 4, DSUB))
            bg.append(lambda: qproj_part(2, 1, 0, 4))
            bg.append(lambda: qproj_part(2, 1, 4, DSUB))
            bg.append(lambda: qproj_part(3, 1, 0, 4))
            bg.append(lambda: qproj_part(3, 1, 4, DSUB))

            # exp stream: half 0
            for h in range(HL):
                for kt in range(KT):
                    scores(h, kt, 0)
                    pump(1)
                push_pv(h, 0)
                push_pv(h, 1)
            for st in range(8):
                push_phase_c(st)

            # exp stream: half 1 (h3's PV rides inline, lagged one kt)
            for h in range(HL):
                inline = h == HL - 1
                if inline:
                    while bg:  # h3's inline PV holds both psW slots
                        pump(1)
                for kt in range(KT):
                    scores(h, kt, 1)
                    if inline:
                        if kt > 0:
                            pv_part(3, 2, kt - 1, kt)
                            pv_part(3, 3, kt - 1, kt)
                        pump(1)
                    else:
                        pump(2)
                if not inline:
                    push_pv(h, 2)
                    push_pv(h, 3)
                if h == 1:
                    # s2=0 tail partials can run once heads 0/1 are normed
                    for st in range(8, 16):
                        bg.append(lambda st=st: phase_c_a(st, 0))
                        bg.append(lambda st=st: phase_c_a(st, 1))
            pv_part(3, 2, KT - 1, KT)
            pv_part(3, 3, KT - 1, KT)
            while bg:
                pump(1)
            norm_unit(3, 2)
            norm_unit(3, 3)
            tail_eng = [nc.gpsimd, nc.scalar, nc.sync]
            for st in range(8, 16):
                phase_c_b(st, 0, None)
                phase_c_b(st, 1, tail_eng[st % 3])

    return nc


_PROGRAM_CACHE: dict[int, bass.Bass] = {}


def _get_program(C: int) -> bass.Bass:
    if C not in _PROGRAM_CACHE:
        nc = build_program(C)
        nc.finalize()
        _PROGRAM_CACHE[C] = nc
    return _PROGRAM_CACHE[C]


def _ceil128(n: int) -> int:
    return max(128, (n + 127) // 128 * 128)


def prepare_in_maps(qs, mask, Wqkv, Wout):
    """Shard FULL inputs into 8 per-core input maps. Returns (in_maps, C)."""
    import ml_dtypes

    np_mm = ml_dtypes.bfloat16
    qs = np.ascontiguousarray(qs, dtype=np.float32)
    mask = np.asarray(mask)
    Wqkv = np.ascontiguousarray(Wqkv, dtype=np.float32)
    Wout = np.ascontiguousarray(Wout, dtype=np.float32)

    nvalid = [int(np.count_nonzero(mask[b])) for b in range(B)]
    if min(nvalid) == 0:
        C = S  # degenerate masks: run dense
    else:
        C = min(S, _ceil128(max(nvalid)))
    compact = C < S
    KT = C // 128
    NKC = (C + QCH - 1) // QCH

    def tile_x(x, nch):
        # [D, n] -> [nch, 128, DSUB, 512] with zero pad to nch*512 cols
        n = x.shape[1]
        xp = np.zeros((D, nch * QCH), dtype=x.dtype)
        xp[:, :n] = x
        return np.ascontiguousarray(
            xp.reshape(DSUB, 128, nch, QCH).transpose(2, 1, 0, 3)
        )

    xq, xkv, mb = [], [], []
    for b in range(B):
        xqb = qs[b].T.astype(np_mm)  # [D, S]
        xq.append(tile_x(xqb, NQC))
        if compact:
            idx = np.nonzero(mask[b] != 0)[0]
            sel = np.concatenate(
                [idx, np.zeros(C - len(idx), dtype=idx.dtype)]
            )
            bias = np.full(C, NEG, dtype=np.float32)
            bias[: len(idx)] = 0.0
            xkv.append(tile_x(qs[b][sel].T.astype(np_mm), NKC))
        else:
            bias = np.where(mask[b] != 0, 0.0, NEG).astype(np.float32)
            xkv.append(tile_x(xqb, NKC))
        mb.append(np.ascontiguousarray(bias.reshape(KT, 128).T))

    in_maps = []
    for b in range(B):
        for g in range(HG):
            h0 = g * HL
            wq = Wqkv[:, (0 * H + h0) * A : (0 * H + h0 + HL) * A] * (
                1.0 / np.sqrt(np.float32(A))
            )
            wk = Wqkv[:, (1 * H + h0) * A : (1 * H + h0 + HL) * A]
            wv = Wqkv[:, (2 * H + h0) * A : (2 * H + h0 + HL) * A]
            # [3, 128, DSUB, 256]: block-major, per-partition contiguous
            wqkv_s = np.ascontiguousarray(
                np.stack([wq, wk, wv])
                .astype(np_mm)
                .reshape(3, DSUB, 128, HL * A)
                .transpose(0, 2, 1, 3)
            )
            wout_s = np.ascontiguousarray(
                Wout[h0 * A : (h0 + HL) * A, :]
                .astype(np_mm)
                .reshape(2, 128, D)
                .transpose(1, 0, 2)
            )
            in_maps.append(
                {
                    "xq": xq[b],
                    "xkv": xkv[b],
                    "wqkv": wqkv_s,
                    "wout": wout_s,
                    "mbias": mb[b],
                }
            )
    return in_maps, C


def gather_output(results, bout):
    """Sum the 4 head-group partials per batch and add bout."""
    out = np.empty((B, S, D), dtype=np.float32)
    for b in range(B):
        acc = results[b * HG]["out"].astype(np.float32).copy()
        for g in range(1, HG):
            acc += results[b * HG + g]["out"]
        out[b] = acc + bout.astype(np.float32)[None, :]
    return out


def _ensure_ntff_hook():
    """Inject antenv.axon_hooks (missing on this image) so trace=True works."""
    import sys
    import types

    try:
        from antenv import axon_hooks  # noqa: F401
        return
    except ImportError:
        pass
    mod = types.ModuleType("antenv.axon_hooks")
    _h = [None]
    mod.set_axon_ntff_profile_hook = lambda h: _h.__setitem__(0, h)
    mod.get_axon_ntff_profile_hook = lambda: _h[0]
    sys.modules["antenv.axon_hooks"] = mod
    import antenv

    antenv.axon_hooks = mod
    try:
        from trn_agent_boot.trn_boot import _ntff_profile_via_ctypes

        mod.set_axon_ntff_profile_hook(
            _ntff_profile_via_ctypes("/opt/axon/libaxon_pjrt.so")
        )
    except Exception:
        pass


def run(qs, mask, Wqkv, Wout, bout, trace=False):
    if trace:
        _ensure_ntff_hook()
    in_maps, C = prepare_in_maps(qs, mask, Wqkv, Wout)
    nc = _get_program(C)
    res = run_bass_kernel_spmd(
        nc, in_maps, core_ids=list(range(B * HG)), trace=trace
    )
    return gather_output(res.results, np.asarray(bout)), res


def kernel(qs, mask, Wqkv, Wout, bout):
    return run(qs, mask, Wqkv, Wout, bout, trace=False)[0]


# revision 31
# speedup vs baseline: 1.0800x; 1.0800x over previous
"""Multi-head attention (B=2, S=2048, D=1024, H=16, A=64) on 8 TRN2 NeuronCores.

Sharding: core c = b*4 + g handles batch b and head-group g (4 heads).
 - Tensor-parallel over heads; host-side all-reduce of the 4 partial output
   projections per batch during the gather.
 - Key/value sequence is mask-compacted host-side (padded to 128; pad slots
   get an additive -60 bias before exp).

v3 schedule (vs the phase-sequential v1 at ~190us):
 - Both the scalar-engine exp stream (72 x [128,1024] ACTIVATEs ~= 82us) and
   the tensor engine (~89us busy) are near-critical, so emission interleaves
   at sub-microsecond granularity: after each scores+exp pair, a background
   queue doles out ~0.5us of deferred tensor work (projection halves, PV
   thirds, out-projection blocks). Out-projection for the first half of the
   sequence runs while the second half's exps stream.
 - All inputs are host-pre-tiled so every DMA is per-partition contiguous,
   and they load on 3 parallel queues; the first matmul starts ~2us in.
 - Softmax denominator: reciprocal on DVE, partition-broadcast via a DRAM
   round-trip (SBUF->DRAM->SBUF with a 0-stride partition read), the one
   pattern proven reliable on hardware. The PV psum is drained once to SBUF
   so the psum slot frees quickly.
 - PSUM: scores 2x[128,1024] (4 banks) + shared proj/PV pool 2 + outproj 2.
"""

import numpy as np

import concourse.bass as bass
import concourse.bacc as bacc
import concourse.mybir as mybir
import concourse.tile as tile
from concourse.bass_utils import run_bass_kernel_spmd

F32 = mybir.dt.float32
BF16 = mybir.dt.bfloat16
DT_MM = BF16
EXP = mybir.ActivationFunctionType.Exp

B = 2
S = 2048
D = 1024
H = 16
A = 64
HG = 4           # head groups (cores per batch)
HL = H // HG     # heads per core = 4
DSUB = D // 128  # 8
NEG = -60.0      # additive mask bias; exp(-60) vanishes in fp32 sums
QCH = 512
NQC = S // QCH   # 4


def build_program(C: int) -> bass.Bass:
    KT = C // 128
    NKC = (C + QCH - 1) // QCH

    nc = bacc.Bacc("TRN2", target_bir_lowering=False, name=f"mha3_c{C}")
    # host-pre-tiled layouts: every DMA below is per-partition contiguous
    xq_d = nc.dram_tensor("xq", [NQC, 128, DSUB, QCH], DT_MM,
                          kind="ExternalInput")
    xkv_d = nc.dram_tensor("xkv", [NKC, 128, DSUB, QCH], DT_MM,
                           kind="ExternalInput")
    wqkv_d = nc.dram_tensor("wqkv", [3, 128, DSUB, HL * A], DT_MM,
                            kind="ExternalInput")
    wout_d = nc.dram_tensor("wout", [128, 2, D], DT_MM, kind="ExternalInput")
    mb_d = nc.dram_tensor("mbias", [128, KT], F32, kind="ExternalInput")
    out_d = nc.dram_tensor("out", [S, D], DT_MM, kind="ExternalOutput")

    with tile.TileContext(nc) as tc:
        with (
            tc.tile_pool(name="const", bufs=1) as const,
            tc.tile_pool(name="xstrq", bufs=4) as xstrq,
            tc.tile_pool(name="xstrk", bufs=3) as xstrk,
            tc.tile_pool(name="probs", bufs=24) as probs,
            tc.tile_pool(name="norm", bufs=3) as norm,
            tc.tile_pool(name="outp", bufs=4) as outp,
            tc.tile_pool(name="dramp", bufs=4, space="DRAM") as dramp,
            tc.tile_pool(name="psS", bufs=2, space="PSUM") as psS,
            tc.tile_pool(name="psW", bufs=2, space="PSUM") as psW,
            tc.tile_pool(name="psO", bufs=2, space="PSUM") as psO,
        ):
            # ---- persistent SBUF residents ----
            w_sb = const.tile([128, 3, DSUB, HL * A], DT_MM)
            wout_sb = const.tile([128, 2, D], DT_MM)
            mb_sb = const.tile([128, KT], F32)
            qT = const.tile([128, 2, S], DT_MM)       # [hp*64+a, h2, qi]
            kT = const.tile([128, 2, C], DT_MM)       # [hp*64+a, h2, ki]
            vx = const.tile([128, KT, HL, A + 1], DT_MM)  # [ki%128, kt, h, a|1]
            ctxT = const.tile([128, 2, S], DT_MM)     # [hp*64+a, h2, qi]
            ones_col = const.tile([128, KT, HL, 1], F32)

            # ---- parallel DMA kickoff (sync + gpsimd queues) ----
            xqt = [xstrq.tile([128, DSUB, QCH], DT_MM, tag="xq", name=f"xq{c}")
                   for c in range(NQC)]
            xkt = [xstrk.tile([128, DSUB, QCH], DT_MM, tag="xk", name=f"xk{c}")
                   for c in range(NKC)]
            nc.sync.dma_start(w_sb[:, 0], wqkv_d.ap()[0])
            nc.gpsimd.dma_start(xkt[0], xkv_d.ap()[0])
            nc.sync.dma_start(xqt[0], xq_d.ap()[0])
            nc.gpsimd.dma_start(w_sb[:, 1], wqkv_d.ap()[1])
            nc.sync.dma_start(xqt[1], xq_d.ap()[1])
            nc.gpsimd.dma_start(xkt[1], xkv_d.ap()[1])
            nc.sync.dma_start(w_sb[:, 2], wqkv_d.ap()[2])
            nc.gpsimd.dma_start(mb_sb, mb_d.ap())
            nc.sync.dma_start(xqt[2], xq_d.ap()[2])
            if NKC > 2:
                nc.gpsimd.dma_start(xkt[2], xkv_d.ap()[2])
            nc.sync.dma_start(xqt[3], xq_d.ap()[3])
            nc.gpsimd.dma_start(wout_sb, wout_d.ap())

            nc.vector.memset(ones_col, 1.0)
            nc.vector.tensor_copy(out=vx[:, :, :, A : A + 1], in_=ones_col)

            prs: dict[tuple, list] = {}

            # ----- background tensor-work queue (thunks of ~0.2-0.9us) ----
            bg: list = []

            def pump(n=1):
                for _ in range(n):
                    if bg:
                        bg.pop(0)()

            def scores(h, kt, half):
                hp, h2 = h % 2, h // 2
                rows = slice(hp * 64, hp * 64 + 64)
                sc = psS.tile([128, 1024], F32, tag="sc",
                              name=f"sc{h}_{kt}_{half}")
                for cc in range(2):
                    q0 = half * 1024 + cc * QCH
                    nc.tensor.matmul(
                        sc[:, cc * QCH : (cc + 1) * QCH],
                        kT[rows, h2, kt * 128 : (kt + 1) * 128],
                        qT[rows, h2, q0 : q0 + QCH],
                        start=True,
                        stop=True,
                    )
                pr = probs.tile([128, 1024], DT_MM, tag="pr",
                                name=f"pr{h}_{kt}_{half}")
                nc.scalar.activation(
                    out=pr, in_=sc, func=EXP,
                    bias=mb_sb[:, kt : kt + 1], scale=1.0,
                )
                prs.setdefault((h, half), []).append(pr)

            _proj_tiles: dict = {}

            def qproj_part(c, ct, lo, hi):
                key = ("q", c, ct)
                if key not in _proj_tiles:
                    _proj_tiles[key] = psW.tile([128, QCH], F32, tag="w",
                                                name=f"qps{c}_{ct}")
                ps = _proj_tiles[key]
                for o in range(lo, hi):
                    nc.tensor.matmul(
                        ps,
                        w_sb[:, 0, o, ct * 128 : (ct + 1) * 128],
                        xqt[c][:, o, :],
                        start=(o == 0),
                        stop=(o == DSUB - 1),
                    )
                if hi == DSUB:
                    nc.vector.tensor_copy(
                        out=qT[:, ct, c * QCH : (c + 1) * QCH], in_=ps
                    )

            def kproj_part(c, ct, lo, hi):
                w = min(QCH, C - c * QCH)
                key = ("k", c, ct)
                if key not in _proj_tiles:
                    _proj_tiles[key] = psW.tile([128, QCH], F32, tag="w",
                                                name=f"kps{c}_{ct}")
                ps = _proj_tiles[key]
                for o in range(lo, hi):
                    nc.tensor.matmul(
                        ps[:, :w],
                        w_sb[:, 1, o, ct * 128 : (ct + 1) * 128],
                        xkt[c][:, o, :w],
                        start=(o == 0),
                        stop=(o == DSUB - 1),
                    )
                if hi == DSUB:
                    nc.vector.tensor_copy(
                        out=kT[:, ct, c * QCH : c * QCH + w], in_=ps[:, :w]
                    )

            def vproj(kt):
                c, k4 = kt // 4, kt % 4
                ps = psW.tile([128, QCH], F32, tag="w", name=f"vps{kt}")
                pvv = ps[:, 0 : HL * A]
                for o in range(DSUB):
                    nc.tensor.matmul(
                        pvv,
                        xkt[c][:, o, k4 * 128 : (k4 + 1) * 128],
                        w_sb[:, 2, o, :],
                        start=(o == 0),
                        stop=(o == DSUB - 1),
                    )
                nc.vector.tensor_copy(
                    out=vx[:, kt, :, 0:A],
                    in_=pvv.rearrange("p (h a) -> p h a", a=A),
                )

            _pv_tiles: dict = {}

            def pv_part(h, c, klo, khi):
                half, sub = c // 2, (c % 2) * QCH
                key = (h, c)
                if key not in _pv_tiles:
                    _pv_tiles[key] = psW.tile([128, QCH], F32, tag="w",
                                              name=f"pv{h}_{c}")
                pv = _pv_tiles[key]
                pl = prs[(h, half)]
                for kt in range(klo, khi):
                    nc.tensor.matmul(
                        pv[0 : A + 1, :],
                        vx[:, kt, h, :],
                        pl[kt][:, sub : sub + QCH],
                        start=(kt == 0),
                        stop=(kt == KT - 1),
                    )

            def norm_unit(h, c):
                hp, h2 = h % 2, h // 2
                pv = _pv_tiles[(h, c)]
                cslice = slice(c * QCH, (c + 1) * QCH)
                ctxr = norm.tile([A + 1, QCH], F32, tag="ctxr")
                nc.vector.tensor_copy(out=ctxr, in_=pv[0 : A + 1, :])
                dnd = dramp.tile([1, QCH], F32, tag="dnd")
                nc.gpsimd.dma_start(dnd, ctxr[A : A + 1, :])
                rBr = norm.tile([64, QCH], F32, tag="rBr")
                dnd_b = bass.AP(
                    tensor=dnd.tensor,
                    offset=dnd.offset,
                    ap=[[0, 64], list(dnd.ap[1])],
                )
                nc.gpsimd.dma_start(rBr, dnd_b)
                rB = norm.tile([64, QCH], F32, tag="rB")
                nc.vector.reciprocal_approx_fast(rB, rBr)
                if hp == 0:
                    nc.vector.tensor_tensor(
                        ctxT[0:64, h2, cslice], ctxr[0:A, :], rB,
                        mybir.AluOpType.mult,
                    )
                else:
                    stg = norm.tile([64, QCH], DT_MM, tag="stg")
                    nc.vector.tensor_tensor(
                        stg, ctxr[0:A, :], rB, mybir.AluOpType.mult
                    )
                    nc.gpsimd.dma_start(ctxT[64:128, h2, cslice], stg)

            def push_pv(h, c):
                bg.append(lambda: pv_part(h, c, 0, 3))
                bg.append(lambda: pv_part(h, c, 3, 6))

                def last():
                    pv_part(h, c, 6, KT)
                    norm_unit(h, c)
                bg.append(last)

            def phase_c(st, dc, dma_eng, ot_holder):
                if dc == 0:
                    ot_holder[st] = outp.tile([128, D], DT_MM, tag="ot",
                                              name=f"ot{st}")
                ot = ot_holder[st]
                po = psO.tile([128, QCH], F32, tag="po", name=f"po{st}_{dc}")
                for s2 in range(2):
                    nc.tensor.matmul(
                        po,
                        ctxT[:, s2, st * 128 : (st + 1) * 128],
                        wout_sb[:, s2, dc * QCH : (dc + 1) * QCH],
                        start=(s2 == 0),
                        stop=(s2 == 1),
                    )
                nc.vector.tensor_copy(
                    out=ot[:, dc * QCH : (dc + 1) * QCH], in_=po
                )
                if dc == 1:
                    dma_eng.dma_start(
                        out_d.ap()[st * 128 : (st + 1) * 128, :], ot
                    )

            _ot: dict = {}

            def push_phase_c(st):
                bg.append(lambda: phase_c(st, 0, nc.sync, _ot))
                bg.append(lambda: phase_c(st, 1, nc.sync, _ot))

            def phase_c_a(st, dc):
                # s2=0 partial (heads 0/1) for the tail sts; runs mid-stream
                if dc == 0:
                    _ot[st] = outp.tile([128, D], DT_MM, tag="ot",
                                        name=f"ot{st}")
                ot = _ot[st]
                po = psO.tile([128, QCH], F32, tag="po", name=f"poa{st}_{dc}")
                nc.tensor.matmul(
                    po,
                    ctxT[:, 0, st * 128 : (st + 1) * 128],
                    wout_sb[:, 0, dc * QCH : (dc + 1) * QCH],
                    start=True,
                    stop=True,
                )
                nc.vector.tensor_copy(
                    out=ot[:, dc * QCH : (dc + 1) * QCH], in_=po
                )

            def phase_c_b(st, dc, dma_eng):
                # s2=1 partial (heads 2/3) + add + store: the exposed tail
                ot = _ot[st]
                po = psO.tile([128, QCH], F32, tag="po", name=f"pob{st}_{dc}")
                nc.tensor.matmul(
                    po,
                    ctxT[:, 1, st * 128 : (st + 1) * 128],
                    wout_sb[:, 1, dc * QCH : (dc + 1) * QCH],
                    start=True,
                    stop=True,
                )
                osl = ot[:, dc * QCH : (dc + 1) * QCH]
                nc.vector.tensor_tensor(osl, po, osl, mybir.AluOpType.add)
                if dc == 1:
                    dma_eng.dma_start(
                        out_d.ap()[st * 128 : (st + 1) * 128, :], ot
                    )

            # ---------------- emission ----------------
            # pre-stream: minimum to unblock h0 half-0 scores
            qproj_part(0, 0, 0, 4)
            qproj_part(0, 0, 4, DSUB)
            qproj_part(1, 0, 0, 4)
            qproj_part(1, 0, 4, DSUB)
            kproj_part(0, 0, 0, 4)
            kproj_part(0, 0, 4, DSUB)

            # background work, dependency-ordered
            bg.append(lambda: kproj_part(1, 0, 0, 4))
            bg.append(lambda: kproj_part(1, 0, 4, DSUB))
            if NKC > 2:
                bg.append(lambda: kproj_part(2, 0, 0, DSUB))
            bg.append(lambda: qproj_part(0, 1, 0, 4))
            bg.append(lambda: qproj_part(0, 1, 4, DSUB))
            bg.append(lambda: qproj_part(1, 1, 0, 4))
            bg.append(lambda: qproj_part(1, 1, 4, DSUB))
            for kt in range(KT):
                bg.append(lambda kt=kt: vproj(kt))
            for c in range(NKC):
                bg.append(lambda c=c: kproj_part(c, 1, 0, 4))
                bg.append(lambda c=c: kproj_part(c, 1, 4, DSUB))
            bg.append(lambda: qproj_part(2, 0, 0, 4))
            bg.append(lambda: qproj_part(2, 0, 4, DSUB))
            bg.append(lambda: qproj_part(3, 0, 0, 4))
            bg.append(lambda: qproj_part(3, 0, 4, DSUB))
            bg.append(lambda: qproj_part(2, 1, 0, 4))
            bg.append(lambda: qproj_part(2, 1, 4, DSUB))
            bg.append(lambda: qproj_part(3, 1, 0, 4))
            bg.append(lambda: qproj_part(3, 1, 4, DSUB))

            # exp stream: half 0
            for h in range(HL):
                for kt in range(KT):
                    scores(h, kt, 0)
                    pump(1)
                push_pv(h, 0)
                push_pv(h, 1)
            for st in range(8):
                push_phase_c(st)

            # exp stream: half 1 (h3's PV rides inline, lagged one kt)
            for h in range(HL):
                inline = h == HL - 1
                if inline:
                    while bg:  # h3's inline PV holds both psW slots
                        pump(1)
                for kt in range(KT):
                    scores(h, kt, 1)
                    if inline:
                        if kt > 0:
                            pv_part(3, 2, kt - 1, kt)
                            pv_part(3, 3, kt - 1, kt)
                        pump(1)
                    else:
                        pump(2)
                if not inline:
                    push_pv(h, 2)
                    push_pv(h, 3)
            pv_part(3, 2, KT - 1, KT)
            pv_part(3, 3, KT - 1, KT)
            while bg:
                pump(1)
            norm_unit(3, 2)
            norm_unit(3, 3)
            for st in range(8, 16):
                phase_c(st, 0, nc.sync if st % 2 == 0 else nc.gpsimd, _ot)
                phase_c(st, 1, nc.sync if st % 2 == 0 else nc.gpsimd, _ot)

    return nc


_PROGRAM_CACHE: dict[int, bass.Bass] = {}


def _get_program(C: int) -> bass.Bass:
    if C not in _PROGRAM_CACHE:
        nc = build_program(C)
        nc.finalize()
        _PROGRAM_CACHE[C] = nc
    return _PROGRAM_CACHE[C]


def _ceil128(n: int) -> int:
    return max(128, (n + 127) // 128 * 128)


def prepare_in_maps(qs, mask, Wqkv, Wout):
    """Shard FULL inputs into 8 per-core input maps. Returns (in_maps, C)."""
    import ml_dtypes

    np_mm = ml_dtypes.bfloat16
    qs = np.ascontiguousarray(qs, dtype=np.float32)
    mask = np.asarray(mask)
    Wqkv = np.ascontiguousarray(Wqkv, dtype=np.float32)
    Wout = np.ascontiguousarray(Wout, dtype=np.float32)

    nvalid = [int(np.count_nonzero(mask[b])) for b in range(B)]
    if min(nvalid) == 0:
        C = S  # degenerate masks: run dense
    else:
        C = min(S, _ceil128(max(nvalid)))
    compact = C < S
    KT = C // 128
    NKC = (C + QCH - 1) // QCH

    def tile_x(x, nch):
        # [D, n] -> [nch, 128, DSUB, 512] with zero pad to nch*512 cols
        n = x.shape[1]
        xp = np.zeros((D, nch * QCH), dtype=x.dtype)
        xp[:, :n] = x
        return np.ascontiguousarray(
            xp.reshape(DSUB, 128, nch, QCH).transpose(2, 1, 0, 3)
        )

    xq, xkv, mb = [], [], []
    for b in range(B):
        xqb = qs[b].T.astype(np_mm)  # [D, S]
        xq.append(tile_x(xqb, NQC))
        if compact:
            idx = np.nonzero(mask[b] != 0)[0]
            sel = np.concatenate(
                [idx, np.zeros(C - len(idx), dtype=idx.dtype)]
            )
            bias = np.full(C, NEG, dtype=np.float32)
            bias[: len(idx)] = 0.0
            xkv.append(tile_x(qs[b][sel].T.astype(np_mm), NKC))
        else:
            bias = np.where(mask[b] != 0, 0.0, NEG).astype(np.float32)
            xkv.append(tile_x(xqb, NKC))
        mb.append(np.ascontiguousarray(bias.reshape(KT, 128).T))

    in_maps = []
    for b in range(B):
        for g in range(HG):
            h0 = g * HL
            wq = Wqkv[:, (0 * H + h0) * A : (0 * H + h0 + HL) * A] * (
                1.0 / np.sqrt(np.float32(A))
            )
            wk = Wqkv[:, (1 * H + h0) * A : (1 * H + h0 + HL) * A]
            wv = Wqkv[:, (2 * H + h0) * A : (2 * H + h0 + HL) * A]
            # [3, 128, DSUB, 256]: block-major, per-partition contiguous
            wqkv_s = np.ascontiguousarray(
                np.stack([wq, wk, wv])
                .astype(np_mm)
                .reshape(3, DSUB, 128, HL * A)
                .transpose(0, 2, 1, 3)
            )
            wout_s = np.ascontiguousarray(
                Wout[h0 * A : (h0 + HL) * A, :]
                .astype(np_mm)
                .reshape(2, 128, D)
                .transpose(1, 0, 2)
            )
            in_maps.append(
                {
                    "xq": xq[b],
                    "xkv": xkv[b],
                    "wqkv": wqkv_s,
                    "wout": wout_s,
                    "mbias": mb[b],
                }
            )
    return in_maps, C


def gather_output(results, bout):
    """Sum the 4 head-group partials per batch and add bout."""
    out = np.empty((B, S, D), dtype=np.float32)
    for b in range(B):
        acc = results[b * HG]["out"].astype(np.float32).copy()
        for g in range(1, HG):
            acc += results[b * HG + g]["out"]
        out[b] = acc + bout.astype(np.float32)[None, :]
    return out


def _ensure_ntff_hook():
    """Inject antenv.axon_hooks (missing on this image) so trace=True works."""
    import sys
    import types

    try:
        from antenv import axon_hooks  # noqa: F401
        return
    except ImportError:
        pass
    mod = types.ModuleType("antenv.axon_hooks")
    _h = [None]
    mod.set_axon_ntff_profile_hook = lambda h: _h.__setitem__(0, h)
    mod.get_axon_ntff_profile_hook = lambda: _h[0]
    sys.modules["antenv.axon_hooks"] = mod
    import antenv

    antenv.axon_hooks = mod
    try:
        from trn_agent_boot.trn_boot import _ntff_profile_via_ctypes

        mod.set_axon_ntff_profile_hook(
            _ntff_profile_via_ctypes("/opt/axon/libaxon_pjrt.so")
        )
    except Exception:
        pass


def run(qs, mask, Wqkv, Wout, bout, trace=False):
    if trace:
        _ensure_ntff_hook()
    in_maps, C = prepare_in_maps(qs, mask, Wqkv, Wout)
    nc = _get_program(C)
    res = run_bass_kernel_spmd(
        nc, in_maps, core_ids=list(range(B * HG)), trace=trace
    )
    return gather_output(res.results, np.asarray(bout)), res


def kernel(qs, mask, Wqkv, Wout, bout):
    return run(qs, mask, Wqkv, Wout, bout, trace=False)[0]
